# revision 1
# baseline (speedup 1.0000x reference)
"""CapsuleRewardHead Trainium2 kernel (8-core data parallel).

Math (per batch row b):
    primary = x @ W + b_lin                    [B, 128]  (128 = 8 caps x 16 dim)
    u_hat[b,o,i,j] = sum_c primary[b,i,c] * out_caps[o,i,c,j]
    3 rounds of dynamic routing over N=32 capsule pairs (o,i), D=16
    out[b] = |squash(s_final)|

Device strategy per core (2048 batch rows):
  - host: transpose x shard -> xt [4096, 2048] fp32 so the hidden dim lands on
    SBUF partitions (PE contracts over partitions); replicate small params.
  - stream xt in batch-slices of 128 cols over HWDGE; matmuls run in float32r
    (single-pass, ~bf16 rate, ~10+ mantissa bits) so no cast pass is needed.
  - MM1 (PE): primaryT[ic, b] += W[h,ic].T @ xT[h, b] over 32 h-chunks into
    PSUM; the Linear bias rides along as an extra K=1 matmul against ones.
  - MM2 (PE): u_hat[b, (o,i,j)] via block-diagonal capsule matrices straight
    into routing layout [128b, 512]; an extra N=16 matmul against
    sum_o(caps) yields round-0's uniform-coefficient sum t0 for free.
  - routing: batched over descending groups of 128-row chunks (scalar chains
    amortize over early big batches; last-arriving chunks form tiny batches
    for a short tail). Softmax exp runs on ACT with a broadcast (step-0) read
    so the weighted-sum multiply is a unit-stride bf16 2x-mode DVE op;
    agreement multiplies go to GPSIMD; PSUM drains go to ACT Copy (table-free
    next to Exp); sqrt via bit-trick seed + one Heron step on DVE.
    Unnormalized accumulators (q = |t|^2, se = sum e) keep the per-round
    scalar chain short: alpha = sqrt(q)/(se^2+q), out = q/(se^2+q).
"""

import os

import numpy as np
import ml_dtypes

B = 16384
HIDDEN = 4096
NUM_OBJ = 4
NUM_CAPS = 8
CAP_DIM = 16
N_ROUTE = 32  # NUM_OBJ * NUM_CAPS
N_CORES = 8

LAST_EXEC_TIME_NS = None  # set after each run when BASS_TRACE=1

BF16 = ml_dtypes.bfloat16
SQRT_MAGIC = 0x1FBD1DF5


def _ap(ap, dims):
    import concourse.bass as bass

    return bass.AP(tensor=ap.tensor, offset=ap.offset, ap=dims)


def build_bass(hidden=HIDDEN, b_sh=B // N_CORES, batch_plan=(8, 5, 2, 1)):
    import concourse.tile as tile
    from concourse import bacc, mybir

    NH = hidden // 128
    NCH = b_sh // 128  # chunks == supers (128 batch cols each)
    assert sum(batch_plan) == NCH
    N, D = N_ROUTE, CAP_DIM
    dt = mybir.dt
    AX = mybir.AxisListType
    OP = mybir.AluOpType
    AF = mybir.ActivationFunctionType

    batches = []
    pos = 0
    for k in batch_plan:
        batches.append(list(range(pos, pos + k)))
        pos += k
    last_chunk_to_batch = {b[-1]: bi for bi, b in enumerate(batches)}
    chunk_to_batch = {}
    for bi, chs in enumerate(batches):
        for ch in chs:
            chunk_to_batch[ch] = bi

    nc = bacc.Bacc("TRN2", target_bir_lowering=False, debug=False, num_devices=N_CORES)

    xt_ap = nc.dram_tensor("xt", [hidden, b_sh], dt.float32, kind="ExternalInput").ap()
    w_ap = nc.dram_tensor("w", [NH, 128, 128], dt.bfloat16, kind="ExternalInput").ap()
    caps_ap = nc.dram_tensor(
        "caps", [NUM_OBJ, 128, 128], dt.bfloat16, kind="ExternalInput"
    ).ap()
    capsum_ap = nc.dram_tensor(
        "capsum", [128, CAP_DIM], dt.bfloat16, kind="ExternalInput"
    ).ap()
    bias_ap = nc.dram_tensor("bias", [1, 384], dt.bfloat16, kind="ExternalInput").ap()
    out_ap = nc.dram_tensor("out", [b_sh], dt.float32, kind="ExternalOutput").ap()

    def r32(ap):
        return ap.bitcast(dt.float32r)

    with tile.TileContext(nc) as tc:
        with (
            tc.tile_pool(name="singles", bufs=1) as singles,
            tc.tile_pool(name="xs", bufs=2) as xs_pool,
            tc.tile_pool(name="primt", bufs=2) as primt_pool,
            tc.tile_pool(name="batch", bufs=1) as bpool,
            tc.tile_pool(name="tmp", bufs=2) as tmp_pool,
            tc.tile_pool(name="sm", bufs=8) as sm_pool,
            tc.tile_pool(name="psum_p", bufs=2, space="PSUM") as psp_pool,
            tc.tile_pool(name="psum_u", bufs=3, space="PSUM") as psu_pool,
            tc.tile_pool(name="psum_t", bufs=2, space="PSUM") as pst_pool,
        ):
            w_sb = singles.tile([128, NH, 128], dt.bfloat16)
            nc.sync.dma_start(out=w_sb[:], in_=w_ap.rearrange("h p f -> p h f"))
            caps_sb = singles.tile([128, NUM_OBJ, 128], dt.bfloat16)
            nc.sync.dma_start(out=caps_sb[:], in_=caps_ap.rearrange("o p f -> p o f"))
            capsum_sb = singles.tile([128, CAP_DIM], dt.bfloat16)
            nc.sync.dma_start(out=capsum_sb[:], in_=capsum_ap[:, :])
            bias_sb = singles.tile([1, 384], dt.bfloat16)
            nc.sync.dma_start(out=bias_sb[:], in_=bias_ap[:, :])
            magic_sb = singles.tile([128, 1], dt.uint32)
            nc.vector.memset(magic_sb[:], SQRT_MAGIC)
            out_sb = singles.tile([128, NCH], dt.float32)

            xt_v = xt_ap.rearrange("(hc p) b -> p hc b", p=128)

            uh_all, t_all, b_all = {}, {}, {}
            for bi, chs in enumerate(batches):
                K = len(chs)
                uh_all[bi] = bpool.tile(
                    [128, K, N, D], dt.bfloat16, tag=f"uh{bi}", name=f"uh{bi}"
                )
                t_all[bi] = bpool.tile(
                    [128, K, D], dt.float32, tag=f"t{bi}", name=f"t{bi}"
                )
                b_all[bi] = bpool.tile(
                    [128, K, N], dt.float32, tag=f"b{bi}", name=f"b{bi}"
                )

            def smt(K, tag, dtype=dt.float32):
                return sm_pool.tile([128, K], dtype, tag=tag, name=tag)

            def sqrt_half(q, K):
                """bit-trick sqrt seed; error washes out through squash."""
                qu = q.bitcast(dt.uint32)
                s1 = smt(K, "sq1", dt.uint32)
                nc.vector.tensor_single_scalar(
                    s1[:], qu, 1, op=OP.logical_shift_right
                )
                s2 = smt(K, "sq2", dt.uint32)
                nc.vector.tensor_tensor(
                    s2[:],
                    s1[:],
                    _ap(magic_sb[:], [magic_sb[:].ap[0], [0, K]]),
                    op=OP.add,
                )
                return s2.bitcast(dt.float32)  # ~3.5% sqrt approx (validated)

            def routing_batch(bi):
                chs = batches[bi]
                K = len(chs)
                uh = uh_all[bi]
                uh_flat = uh.rearrange("p k n d -> p (k n d)")
                tt = t_all[bi]
                for r in range(3):
                    if r > 0:
                        if r == 2:
                            # r2 logits can reach ~56; subtract the max so
                            # se^2 stays in fp32 range. r1 logits are <~33
                            # (se^2 < 7e30), so r1 exps directly.
                            mx = smt(K, "mx")
                            nc.vector.tensor_reduce(
                                mx[:], b_all[bi][:], axis=AX.X, op=OP.max
                            )
                            bsub = sm_pool.tile(
                                [128, K, N], dt.float32, tag="bsub", name="bsub"
                            )
                            nc.vector.tensor_tensor(
                                bsub[:],
                                b_all[bi][:],
                                _ap(mx[:], [*mx[:].ap, [0, N]]),
                                op=OP.subtract,
                            )
                            esrc = bsub[:]
                        else:
                            esrc = b_all[bi][:]
                        erep = tmp_pool.tile(
                            [128, K, N, D], dt.bfloat16, tag="erep", name="erep"
                        )
                        nc.scalar.activation(
                            erep[:], _ap(esrc, [*esrc.ap, [0, D]]), AF.Exp
                        )
                        se = smt(K, "se")
                        nc.vector.tensor_reduce(
                            se[:],
                            erep[:, :, :, 0:1].rearrange("p k n d -> p k d n"),
                            axis=AX.X,
                            op=OP.add,
                        )
                        wmul = tmp_pool.tile(
                            [128, K, N, D], dt.bfloat16, tag="wmul", name="wmul"
                        )
                        nc.vector.tensor_tensor(
                            wmul.rearrange("p k n d -> p (k n d)"),
                            uh_flat,
                            erep.rearrange("p k n d -> p (k n d)"),
                            op=OP.mult,
                        )
                        nc.vector.tensor_reduce(
                            tt[:],
                            wmul.rearrange("p k n d -> p k d n"),
                            axis=AX.X,
                            op=OP.add,
                        )
                    # q = |t|^2, den = se^2 + q, rden = 1/den
                    sq = sm_pool.tile([128, K, D], dt.float32, tag="sqv", name="sqv")
                    nc.vector.tensor_tensor(sq[:], tt[:], tt[:], op=OP.mult)
                    q = smt(K, "q")
                    nc.vector.tensor_reduce(q[:], sq[:], axis=AX.X, op=OP.add)
                    den = smt(K, "den")
                    if r == 0:
                        nc.vector.tensor_single_scalar(
                            den[:], q[:], float(N * N), op=OP.add
                        )
                    else:
                        se2 = smt(K, "se2")
                        nc.vector.tensor_mul(se2[:], se[:], se[:])
                        nc.vector.tensor_add(den[:], q[:], se2[:])

                    rden = smt(K, "rden")
                    nc.vector.reciprocal(rden[:], den[:])
                    if r < 2:
                        sm = sqrt_half(q[:], K)
                        alpha2 = smt(K, "alpha2")
                        nc.vector.tensor_mul(alpha2[:], sm, rden[:])
                        # replicate t across n on ACT (table-free Copy with
                        # step-0 read) so the agreement multiply runs in
                        # DVE 2x mode on unit-stride bf16
                        trep = tmp_pool.tile(
                            [128, K, N, D], dt.bfloat16, tag="trep", name="trep"
                        )
                        tt3 = tt[:]
                        nc.scalar.copy(
                            trep[:],
                            _ap(tt3, [tt3.ap[0], tt3.ap[1], [0, N], tt3.ap[2]]),
                        )
                        tmp2 = tmp_pool.tile(
                            [128, K, N, D], dt.bfloat16, tag="amul", name="amul"
                        )
                        nc.vector.tensor_tensor(
                            tmp2.rearrange("p k n d -> p (k n d)"),
                            uh_flat,
                            trep.rearrange("p k n d -> p (k n d)"),
                            op=OP.mult,
                        )
                        dta = sm_pool.tile(
                            [128, K, N], dt.bfloat16, tag="dta", name="dta"
                        )
                        with nc.allow_low_precision(reason="dta bf16 validated"):
                            nc.vector.tensor_reduce(
                                dta[:], tmp2[:], axis=AX.X, op=OP.add
                            )
                        if r == 0:
                            nc.vector.tensor_tensor(
                                b_all[bi][:],
                                dta[:],
                                _ap(alpha2[:], [*alpha2[:].ap, [0, N]]),
                                op=OP.mult,
                            )
                        else:
                            badd = sm_pool.tile(
                                [128, K, N], dt.float32, tag="badd", name="badd"
                            )
                            nc.vector.tensor_tensor(
                                badd[:],
                                dta[:],
                                _ap(alpha2[:], [*alpha2[:].ap, [0, N]]),
                                op=OP.mult,
                            )
                            nc.vector.tensor_tensor(
                                b_all[bi][:], b_all[bi][:], badd[:], op=OP.add
                            )
                    else:
                        nc.vector.tensor_mul(
                            out_sb[:, chs[0] : chs[0] + K], q[:], rden[:]
                        )
                        nc.sync.dma_start(
                            out=out_ap.rearrange("(c p) -> p c", p=128)[
                                :, chs[0] : chs[0] + K
                            ],
                            in_=out_sb[:, chs[0] : chs[0] + K],
                        )

            SUP = 512
            CPS = SUP // 128
            NQ = min(8, NH)  # h-slice sub-DMAs per super
            HQ = NH // NQ
            for sp in range(b_sh // SUP):
                xs = xs_pool.tile([128, NH, SUP], dt.bfloat16)
                for qd in range(NQ):
                    nc.gpsimd.dma_start(
                        out=xs[:, qd * HQ : (qd + 1) * HQ, :],
                        in_=xt_v[:, qd * HQ : (qd + 1) * HQ, sp * SUP : (sp + 1) * SUP],
                    )
                psp = psp_pool.tile([128, SUP], dt.float32)
                ones_bc = _ap(
                    bias_sb[:, 128:256], [bias_sb[:, 128:256].ap[0], [0, CPS], [1, 128]]
                )
                if sp == 0:
                    # HAM warmup: zero-contribution streams while x loads
                    zeros_bc = _ap(
                        bias_sb[:, 256:384],
                        [bias_sb[:, 256:384].ap[0], [0, CPS], [1, 128]],
                    )
                    for wi in range(24):
                        nc.tensor.matmul(
                            psp[:], bias_sb[:, 256:384], zeros_bc,
                            start=(wi == 0), stop=False,
                        )
                nc.tensor.matmul(
                    psp[:],
                    bias_sb[:, 0:128],
                    ones_bc,
                    start=(sp != 0),
                    stop=False,
                )
                for h in range(NH):
                    nc.tensor.matmul(
                        psp[:],
                        w_sb[:, h, :],
                        xs[:, h, :],
                        start=False,
                        stop=(h == NH - 1),
                    )
                primt = primt_pool.tile([128, SUP], dt.bfloat16)
                nc.scalar.copy(primt[:], psp[:])

                for c in range(CPS):
                    s = sp * CPS + c
                    bi = chunk_to_batch[s]
                    k = s - batches[bi][0]
                    lhsT = primt[:, c * 128 : (c + 1) * 128]
                    psu = psu_pool.tile([128, NUM_OBJ * 128], dt.float32)
                    nc.tensor.matmul(
                        psu[:],
                        lhsT,
                        caps_sb.rearrange("p o f -> p (o f)"),
                        start=True,
                        stop=True,
                    )
                    pst = pst_pool.tile([128, CAP_DIM], dt.float32)
                    nc.tensor.matmul(
                        pst[:], lhsT, capsum_sb[:], start=True, stop=True
                    )
                    nc.scalar.copy(
                        uh_all[bi][:, k, :, :].rearrange("p n d -> p (n d)"), psu[:]
                    )
                    nc.scalar.copy(t_all[bi][:, k, :], pst[:])

                    if s in last_chunk_to_batch:
                        routing_batch(last_chunk_to_batch[s])



    nc.compile()
    return nc


def _prep_params(W, b_lin, out_caps, hidden=HIDDEN):
    NH = hidden // 128
    w_f = np.ascontiguousarray(
        W.astype(np.float32).reshape(NH, 128, NUM_CAPS * CAP_DIM)
    ).astype(BF16)
    caps_bd = np.zeros((NUM_OBJ, 128, 128), np.float32)
    for o in range(NUM_OBJ):
        for i in range(NUM_CAPS):
            caps_bd[
                o, i * CAP_DIM : (i + 1) * CAP_DIM, i * CAP_DIM : (i + 1) * CAP_DIM
            ] = out_caps[o, i]
    capsum = caps_bd.sum(0)
    caps_bd = caps_bd.astype(BF16)
    capsum_t0 = np.zeros((128, CAP_DIM), np.float32)
    for i in range(NUM_CAPS):
        capsum_t0[i * CAP_DIM : (i + 1) * CAP_DIM, :] = capsum[
            i * CAP_DIM : (i + 1) * CAP_DIM, i * CAP_DIM : (i + 1) * CAP_DIM
        ]
    bias_row = np.concatenate(
        [
            b_lin.astype(np.float32).reshape(1, 128),
            np.ones((1, 128), np.float32),
            np.zeros((1, 128), np.float32),
        ],
        axis=1,
    )
    return (
        w_f,
        caps_bd,
        np.ascontiguousarray(capsum_t0).astype(BF16),
        bias_row.astype(BF16),
    )


_NC_CACHE = {}


def kernel(x, W, b_lin, out_caps):
    global LAST_EXEC_TIME_NS
    from concourse.bass_utils import run_bass_kernel_spmd

    x = np.asarray(x)
    W = np.asarray(W)
    b_lin = np.asarray(b_lin)
    out_caps = np.asarray(out_caps)
    bsz, hidden = x.shape
    b_sh = bsz // N_CORES

    key = (hidden, b_sh)
    if key not in _NC_CACHE:
        _NC_CACHE[key] = build_bass(hidden=hidden, b_sh=b_sh)
    nc = _NC_CACHE[key]

    w_f, caps_bd, capsum_t0, bias_row = _prep_params(W, b_lin, out_caps, hidden)

    in_maps = []
    for i in range(N_CORES):
        shard = x[i * b_sh : (i + 1) * b_sh]
        xt = np.ascontiguousarray(shard.T)  # [hidden, b_sh]
        in_maps.append(
            {
                "xt": xt,
                "w": w_f,
                "caps": caps_bd,
                "capsum": capsum_t0,
                "bias": bias_row,
            }
        )

    res = run_bass_kernel_spmd(
        nc,
        in_maps,
        core_ids=list(range(N_CORES)),
        trace=bool(int(os.environ.get("BASS_TRACE", "0") or "0")),
    )
    LAST_EXEC_TIME_NS = res.exec_time_ns
    return np.concatenate([res.results[i]["out"] for i in range(N_CORES)])



# revision 8
# speedup vs baseline: 1.2840x; 1.2840x over previous
"""CapsuleRewardHead Trainium2 kernel (8-core data parallel).

Math (per batch row b):
    primary = x @ W + b_lin                    [B, 128]  (128 = 8 caps x 16 dim)
    u_hat[b,o,i,j] = sum_c primary[b,i,c] * out_caps[o,i,c,j]
    3 rounds of dynamic routing over N=32 capsule pairs (o,i), D=16
    out[b] = |squash(s_final)|

Device strategy per core (2048 batch rows):
  - host: quantize x shard to fp8 e4m3 and pre-tile to [4 supers][128 part]
    [32 hc][512 b] so each partition's super-payload is contiguous (2KB+
    DMA descriptors); W to fp8 scaled by 1024 (descaled via caps/capsum),
    b_lin rides as a per-partition bias in the PSUM drain.
  - MM1 (PE): DoubleRow fp8 matmuls contract h-chunk PAIRS (256 rows/pass)
    into PSUM: primaryT[ic, b] for each 512-col super.
  - MM2 (PE): u_hat[b, (o,i,j)] via block-diagonal capsule matrices straight
    into routing layout [128b, 512]; an extra N=16 matmul against
    sum_o(caps) yields round-0's uniform-coefficient sum t0 for free.
  - routing: batched over descending groups of 128-row chunks. Softmax exp
    runs on ACT with a broadcast (step-0) read so the weighted-sum multiply
    is a unit-stride bf16 2x-mode DVE op; PSUM drains go to ACT Copy;
    sqrt via bit-trick seed. Unnormalized accumulators (q = |t|^2, se =
    sum e) keep the per-round scalar chain short.
"""

import os

import numpy as np
import ml_dtypes

B = 16384
HIDDEN = 4096
NUM_OBJ = 4
NUM_CAPS = 8
CAP_DIM = 16
N_ROUTE = 32  # NUM_OBJ * NUM_CAPS
N_CORES = 8

LAST_EXEC_TIME_NS = None  # set after each run when BASS_TRACE=1

BF16 = ml_dtypes.bfloat16
FP8 = ml_dtypes.float8_e4m3
W_SCALE = 1024.0
SQRT_MAGIC = 0x1FBD1DF5


def _ap(ap, dims):
    import concourse.bass as bass

    return bass.AP(tensor=ap.tensor, offset=ap.offset, ap=dims)


def build_bass(hidden=HIDDEN, b_sh=B // N_CORES, batch_plan=(8, 5, 2, 1)):
    import concourse.tile as tile
    from concourse import bacc, mybir

    NH = hidden // 128
    NCH = b_sh // 128  # 128-row chunks
    SUP = 512
    NSUP = b_sh // SUP
    CPS = SUP // 128
    assert sum(batch_plan) == NCH
    N, D = N_ROUTE, CAP_DIM
    dt = mybir.dt
    AX = mybir.AxisListType
    OP = mybir.AluOpType
    AF = mybir.ActivationFunctionType
    PM = mybir.MatmulPerfMode

    batches = []
    pos = 0
    for k in batch_plan:
        batches.append(list(range(pos, pos + k)))
        pos += k
    last_chunk_to_batch = {b[-1]: bi for bi, b in enumerate(batches)}
    chunk_to_batch = {}
    for bi, chs in enumerate(batches):
        for ch in chs:
            chunk_to_batch[ch] = bi

    nc = bacc.Bacc("TRN2", target_bir_lowering=False, debug=False, num_devices=N_CORES)

    xt_ap = nc.dram_tensor(
        "xt", [NSUP, 128, NH, SUP], dt.float8e4, kind="ExternalInput"
    ).ap()
    w_ap = nc.dram_tensor("w", [NH, 128, 128], dt.float8e4, kind="ExternalInput").ap()
    caps_ap = nc.dram_tensor(
        "caps", [NUM_OBJ, 128, 128], dt.bfloat16, kind="ExternalInput"
    ).ap()
    capsum_ap = nc.dram_tensor(
        "capsum", [128, CAP_DIM], dt.bfloat16, kind="ExternalInput"
    ).ap()
    bias_ap = nc.dram_tensor("bias", [1, 256], dt.bfloat16, kind="ExternalInput").ap()
    out_ap = nc.dram_tensor("out", [b_sh], dt.float32, kind="ExternalOutput").ap()

    with tile.TileContext(nc) as tc:
        with (
            tc.tile_pool(name="singles", bufs=1) as singles,
            tc.tile_pool(name="xs", bufs=2) as xs_pool,
            tc.tile_pool(name="primt", bufs=2) as primt_pool,
            tc.tile_pool(name="batch", bufs=1) as bpool,
            tc.tile_pool(name="tmp", bufs=2) as tmp_pool,
            tc.tile_pool(name="sm", bufs=8) as sm_pool,
            tc.tile_pool(name="psum_p", bufs=2, space="PSUM") as psp_pool,
            tc.tile_pool(name="psum_u", bufs=3, space="PSUM") as psu_pool,
            tc.tile_pool(name="psum_t", bufs=2, space="PSUM") as pst_pool,
            tc.tile_pool(name="psum_w", bufs=1, space="PSUM") as psw_pool,
        ):
            w_sb = singles.tile([128, NH, 128], dt.float8e4)
            nc.sync.dma_start(out=w_sb[:], in_=w_ap.rearrange("h p f -> p h f"))
            caps_sb = singles.tile([128, NUM_OBJ, 128], dt.bfloat16)
            nc.sync.dma_start(out=caps_sb[:], in_=caps_ap.rearrange("o p f -> p o f"))
            capsum_sb = singles.tile([128, CAP_DIM], dt.bfloat16)
            nc.sync.dma_start(out=capsum_sb[:], in_=capsum_ap[:, :])
            bias_sb = singles.tile([1, 256], dt.bfloat16)
            nc.sync.dma_start(out=bias_sb[:], in_=bias_ap[:, :])
            magic_sb = singles.tile([128, 1], dt.uint32)
            nc.vector.memset(magic_sb[:], SQRT_MAGIC)
            out_sb = singles.tile([128, NCH], dt.float32)
            warm_sb = singles.tile([128, 2, SUP], dt.float8e4)
            nc.vector.memset(warm_sb.rearrange("p a b -> p (a b)"), 0)

            uh_all, t_all, b_all = {}, {}, {}
            for bi, chs in enumerate(batches):
                K = len(chs)
                uh_all[bi] = bpool.tile(
                    [128, K, N, D], dt.bfloat16, tag=f"uh{bi}", name=f"uh{bi}"
                )
                t_all[bi] = bpool.tile(
                    [128, K, D], dt.float32, tag=f"t{bi}", name=f"t{bi}"
                )
                b_all[bi] = bpool.tile(
                    [128, K, N], dt.float32, tag=f"b{bi}", name=f"b{bi}"
                )

            def smt(K, tag, dtype=dt.float32):
                return sm_pool.tile([128, K], dtype, tag=tag, name=tag)

            def sqrt_half(q, K):
                """bit-trick sqrt seed; error washes out through squash."""
                qu = q.bitcast(dt.uint32)
                s1 = smt(K, "sq1", dt.uint32)
                nc.vector.tensor_single_scalar(
                    s1[:], qu, 1, op=OP.logical_shift_right
                )
                s2 = smt(K, "sq2", dt.uint32)
                nc.vector.tensor_tensor(
                    s2[:],
                    s1[:],
                    _ap(magic_sb[:], [magic_sb[:].ap[0], [0, K]]),
                    op=OP.add,
                )
                return s2.bitcast(dt.float32)  # ~3.5% sqrt approx (validated)

            def routing_batch(bi):
                chs = batches[bi]
                K = len(chs)
                uh = uh_all[bi]
                uh_flat = uh.rearrange("p k n d -> p (k n d)")
                tt = t_all[bi]
                for r in range(3):
                    if r > 0:
                        if r == 2:
                            # r2 logits can reach ~56; subtract the max so
                            # se^2 stays in fp32 range. r1 logits are <~33
                            # (se^2 < 7e30), so r1 exps directly.
                            mx = smt(K, "mx")
                            nc.vector.tensor_reduce(
                                mx[:], b_all[bi][:], axis=AX.X, op=OP.max
                            )
                            bsub = sm_pool.tile(
                                [128, K, N], dt.float32, tag="bsub", name="bsub"
                            )
                            nc.vector.tensor_tensor(
                                bsub[:],
                                b_all[bi][:],
                                _ap(mx[:], [*mx[:].ap, [0, N]]),
                                op=OP.subtract,
                            )
                            esrc = bsub[:]
                        else:
                            esrc = b_all[bi][:]
                        erep = tmp_pool.tile(
                            [128, K, N, D], dt.bfloat16, tag="erep", name="erep"
                        )
                        nc.scalar.activation(
                            erep[:], _ap(esrc, [*esrc.ap, [0, D]]), AF.Exp
                        )
                        se = smt(K, "se")
                        nc.vector.tensor_reduce(
                            se[:],
                            erep[:, :, :, 0:1].rearrange("p k n d -> p k d n"),
                            axis=AX.X,
                            op=OP.add,
                        )
                        wmul = tmp_pool.tile(
                            [128, K, N, D], dt.bfloat16, tag="wmul", name="wmul"
                        )
                        nc.vector.tensor_tensor(
                            wmul.rearrange("p k n d -> p (k n d)"),
                            uh_flat,
                            erep.rearrange("p k n d -> p (k n d)"),
                            op=OP.mult,
                        )
                        nc.vector.tensor_reduce(
                            tt[:],
                            wmul.rearrange("p k n d -> p k d n"),
                            axis=AX.X,
                            op=OP.add,
                        )
                    # q = |t|^2, den = se^2 + q, rden = 1/den
                    sq = sm_pool.tile([128, K, D], dt.float32, tag="sqv", name="sqv")
                    nc.vector.tensor_tensor(sq[:], tt[:], tt[:], op=OP.mult)
                    q = smt(K, "q")
                    nc.vector.tensor_reduce(q[:], sq[:], axis=AX.X, op=OP.add)
                    den = smt(K, "den")
                    if r == 0:
                        nc.vector.tensor_single_scalar(
                            den[:], q[:], float(N * N), op=OP.add
                        )
                    else:
                        se2 = smt(K, "se2")
                        nc.vector.tensor_mul(se2[:], se[:], se[:])
                        nc.vector.tensor_add(den[:], q[:], se2[:])

                    rden = smt(K, "rden")
                    nc.vector.reciprocal(rden[:], den[:])
                    if r < 2:
                        sm = sqrt_half(q[:], K)
                        alpha2 = smt(K, "alpha2")
                        nc.vector.tensor_mul(alpha2[:], sm, rden[:])
                        # replicate t across n on ACT (table-free Copy with
                        # step-0 read) so the agreement multiply runs in
                        # DVE 2x mode on unit-stride bf16
                        trep = tmp_pool.tile(
                            [128, K, N, D], dt.bfloat16, tag="trep", name="trep"
                        )
                        tt3 = tt[:]
                        nc.scalar.copy(
                            trep[:],
                            _ap(tt3, [tt3.ap[0], tt3.ap[1], [0, N], tt3.ap[2]]),
                        )
                        tmp2 = tmp_pool.tile(
                            [128, K, N, D], dt.bfloat16, tag="amul", name="amul"
                        )
                        nc.vector.tensor_tensor(
                            tmp2.rearrange("p k n d -> p (k n d)"),
                            uh_flat,
                            trep.rearrange("p k n d -> p (k n d)"),
                            op=OP.mult,
                        )
                        dta = sm_pool.tile(
                            [128, K, N], dt.bfloat16, tag="dta", name="dta"
                        )
                        with nc.allow_low_precision(reason="dta bf16 validated"):
                            nc.vector.tensor_reduce(
                                dta[:], tmp2[:], axis=AX.X, op=OP.add
                            )
                        if r == 0:
                            nc.vector.tensor_tensor(
                                b_all[bi][:],
                                dta[:],
                                _ap(alpha2[:], [*alpha2[:].ap, [0, N]]),
                                op=OP.mult,
                            )
                        else:
                            badd = sm_pool.tile(
                                [128, K, N], dt.float32, tag="badd", name="badd"
                            )
                            nc.vector.tensor_tensor(
                                badd[:],
                                dta[:],
                                _ap(alpha2[:], [*alpha2[:].ap, [0, N]]),
                                op=OP.mult,
                            )
                            nc.vector.tensor_tensor(
                                b_all[bi][:], b_all[bi][:], badd[:], op=OP.add
                            )
                    else:
                        nc.vector.tensor_mul(
                            out_sb[:, chs[0] : chs[0] + K], q[:], rden[:]
                        )
                        nc.sync.dma_start(
                            out=out_ap.rearrange("(c p) -> p c", p=128)[
                                :, chs[0] : chs[0] + K
                            ],
                            in_=out_sb[:, chs[0] : chs[0] + K],
                        )

            # PE p-state warmup while super 0 streams in
            psw = psw_pool.tile([128, SUP], dt.float32)
            for wi in range(24):
                nc.tensor.matmul(
                    psw[:],
                    warm_sb[:, 0, 0:128],
                    warm_sb[:, 1, :],
                    start=(wi == 0),
                    stop=(wi == 23),
                )

            for sp in range(NSUP):
                xs = xs_pool.tile([128, NH, SUP], dt.float8e4)
                NQ = 16 if sp == 0 else 8
                HQ = NH // NQ
                for qd in range(NQ):
                    nc.gpsimd.dma_start(
                        out=xs[:, qd * HQ : (qd + 1) * HQ, :],
                        in_=xt_ap[sp, :, qd * HQ : (qd + 1) * HQ, :],
                    )
                psp = psp_pool.tile([128, SUP], dt.float32)
                # Linear bias rides as a K=1 bf16 matmul against ones
                ones_bc = _ap(
                    bias_sb[:, 128:256],
                    [bias_sb[:, 128:256].ap[0], [0, CPS], [1, 128]],
                )
                nc.tensor.matmul(
                    psp[:], bias_sb[:, 0:128], ones_bc, start=True, stop=False
                )
                for hp in range(NH // 2):
                    nc.tensor.matmul(
                        psp[:],
                        w_sb[:, 2 * hp : 2 * hp + 2, :],
                        xs[:, 2 * hp : 2 * hp + 2, :],
                        start=False,
                        stop=(hp == NH // 2 - 1),
                        perf_mode=PM.DoubleRow,
                    )
                primt = primt_pool.tile([128, SUP], dt.bfloat16)
                nc.scalar.copy(primt[:], psp[:])

                for c in range(CPS):
                    s = sp * CPS + c
                    bi = chunk_to_batch[s]
                    k = s - batches[bi][0]
                    lhsT = primt[:, c * 128 : (c + 1) * 128]
                    psu = psu_pool.tile([128, NUM_OBJ * 128], dt.float32)
                    nc.tensor.matmul(
                        psu[:],
                        lhsT,
                        caps_sb.rearrange("p o f -> p (o f)"),
                        start=True,
                        stop=True,
                    )
                    pst = pst_pool.tile([128, CAP_DIM], dt.float32)
                    nc.tensor.matmul(
                        pst[:], lhsT, capsum_sb[:], start=True, stop=True
                    )
                    nc.scalar.copy(
                        uh_all[bi][:, k, :, :].rearrange("p n d -> p (n d)"), psu[:]
                    )
                    nc.scalar.copy(t_all[bi][:, k, :], pst[:])

                    if s in last_chunk_to_batch:
                        routing_batch(last_chunk_to_batch[s])

    nc.compile()
    return nc


def _prep_params(W, b_lin, out_caps, hidden=HIDDEN):
    NH = hidden // 128
    w_f = np.ascontiguousarray(
        (W.astype(np.float32) * W_SCALE).reshape(NH, 128, NUM_CAPS * CAP_DIM)
    ).astype(FP8)
    caps_bd = np.zeros((NUM_OBJ, 128, 128), np.float32)
    for o in range(NUM_OBJ):
        for i in range(NUM_CAPS):
            caps_bd[
                o, i * CAP_DIM : (i + 1) * CAP_DIM, i * CAP_DIM : (i + 1) * CAP_DIM
            ] = out_caps[o, i]
    caps_bd /= W_SCALE
    capsum = caps_bd.sum(0)
    caps_bd = caps_bd.astype(BF16)
    capsum_t0 = np.zeros((128, CAP_DIM), np.float32)
    for i in range(NUM_CAPS):
        capsum_t0[i * CAP_DIM : (i + 1) * CAP_DIM, :] = capsum[
            i * CAP_DIM : (i + 1) * CAP_DIM, i * CAP_DIM : (i + 1) * CAP_DIM
        ]
    bias_row = np.concatenate(
        [
            b_lin.astype(np.float32).reshape(1, 128) * W_SCALE,
            np.ones((1, 128), np.float32),
        ],
        axis=1,
    ).astype(BF16)
    return w_f, caps_bd, np.ascontiguousarray(capsum_t0).astype(BF16), bias_row


_NC_CACHE = {}


def kernel(x, W, b_lin, out_caps):
    global LAST_EXEC_TIME_NS
    from concourse.bass_utils import run_bass_kernel_spmd

    x = np.asarray(x)
    W = np.asarray(W)
    b_lin = np.asarray(b_lin)
    out_caps = np.asarray(out_caps)
    bsz, hidden = x.shape
    b_sh = bsz // N_CORES
    NH = hidden // 128
    SUP = 512
    NSUP = b_sh // SUP

    key = (hidden, b_sh)
    if key not in _NC_CACHE:
        _NC_CACHE[key] = build_bass(hidden=hidden, b_sh=b_sh)
    nc = _NC_CACHE[key]

    w_f, caps_bd, capsum_t0, bias_row = _prep_params(W, b_lin, out_caps, hidden)

    in_maps = []
    for i in range(N_CORES):
        shard = x[i * b_sh : (i + 1) * b_sh]
        # [sp, p, hc, b] tiled fp8: contiguous per partition per super
        xt = np.ascontiguousarray(
            shard.reshape(NSUP, SUP, NH, 128).transpose(0, 3, 2, 1)
        ).astype(FP8)
        in_maps.append(
            {
                "xt": xt,
                "w": w_f,
                "caps": caps_bd,
                "capsum": capsum_t0,
                "bias": bias_row,
            }
        )

    res = run_bass_kernel_spmd(
        nc,
        in_maps,
        core_ids=list(range(N_CORES)),
        trace=bool(int(os.environ.get("BASS_TRACE", "0") or "0")),
    )
    LAST_EXEC_TIME_NS = res.exec_time_ns
    return np.concatenate([res.results[i]["out"] for i in range(N_CORES)])


# revision 9
# speedup vs baseline: 1.3645x; 1.0627x over previous
"""CapsuleRewardHead Trainium2 kernel (8-core data parallel).

Math (per batch row b):
    primary = x @ W + b_lin                    [B, 128]  (128 = 8 caps x 16 dim)
    u_hat[b,o,i,j] = sum_c primary[b,i,c] * out_caps[o,i,c,j]
    3 rounds of dynamic routing over N=32 capsule pairs (o,i), D=16
    out[b] = |squash(s_final)|

Device strategy per core (2048 batch rows):
  - host: quantize x shard to fp8 e4m3 and pre-tile to [sp][queue][128 part]
    [hc][b] so every DMA issue reads one fully contiguous DRAM block; W to
    fp8 scaled by 1024 (descaled via caps/capsum); W/caps pre-transposed so
    their loads are contiguous too. Linear bias rides as a K=1 bf16 matmul.
  - MM1 (PE): DoubleRow fp8 matmuls contract h-chunk PAIRS (256 rows/pass)
    into PSUM: primaryT[ic, b] per 512-col super.
  - MM2 (PE): u_hat[b, (o,i,j)] via block-diagonal capsule matrices straight
    into routing layout [128b, 512]; an extra N=16 matmul against
    sum_o(caps) yields round-0's uniform-coefficient sum t0 for free.
  - routing engine split: softmax exp emits compact [K,N] on ACT; ACT also
    materializes the e/t broadcasts so the big multiplies run in DVE 2x
    bf16 mode; round-0's agreement multiply goes to GPSIMD with a
    broadcast t read (GPSIMD is 1x anyway). The n/d reductions run as
    pairwise halving trees of 2x-mode tensor_tensor adds (tensor_reduce
    has no fast mode; trees are ~2x faster). sqrt via bit-trick seed;
    unnormalized accumulators (q = |t|^2, se = sum e) keep the per-round
    scalar chain short.
"""

import os

import numpy as np
import ml_dtypes

B = 16384
HIDDEN = 4096
NUM_OBJ = 4
NUM_CAPS = 8
CAP_DIM = 16
N_ROUTE = 32  # NUM_OBJ * NUM_CAPS
N_CORES = 8

LAST_EXEC_TIME_NS = None  # set after each run when BASS_TRACE=1

BF16 = ml_dtypes.bfloat16
FP8 = ml_dtypes.float8_e4m3
W_SCALE = 1024.0
SQRT_MAGIC = 0x1FBD1DF5
NQ = 8  # x sub-DMA issues per super


def _ap(ap, dims):
    import concourse.bass as bass

    return bass.AP(tensor=ap.tensor, offset=ap.offset, ap=dims)


def build_bass(hidden=HIDDEN, b_sh=B // N_CORES, batch_plan=(6, 4, 3, 2, 1)):
    import concourse.tile as tile
    from concourse import bacc, mybir

    NH = hidden // 128
    NCH = b_sh // 128  # 128-row chunks
    SUP = 512
    NSUP = b_sh // SUP
    CPS = SUP // 128
    HQ = NH // NQ
    assert sum(batch_plan) == NCH
    N, D = N_ROUTE, CAP_DIM
    dt = mybir.dt
    AX = mybir.AxisListType
    OP = mybir.AluOpType
    AF = mybir.ActivationFunctionType
    PM = mybir.MatmulPerfMode

    batches = []
    pos = 0
    for k in batch_plan:
        batches.append(list(range(pos, pos + k)))
        pos += k
    last_chunk_to_batch = {b[-1]: bi for bi, b in enumerate(batches)}
    chunk_to_batch = {}
    for bi, chs in enumerate(batches):
        for ch in chs:
            chunk_to_batch[ch] = bi

    nc = bacc.Bacc("TRN2", target_bir_lowering=False, debug=False, num_devices=N_CORES)

    xt_ap = nc.dram_tensor(
        "xt", [NSUP, NQ, 128, HQ, SUP], dt.float8e4, kind="ExternalInput"
    ).ap()
    w_ap = nc.dram_tensor("w", [128, NH, 128], dt.float8e4, kind="ExternalInput").ap()
    caps_ap = nc.dram_tensor(
        "caps", [128, NUM_OBJ, 128], dt.bfloat16, kind="ExternalInput"
    ).ap()
    capsum_ap = nc.dram_tensor(
        "capsum", [128, CAP_DIM], dt.bfloat16, kind="ExternalInput"
    ).ap()
    bias_ap = nc.dram_tensor("bias", [1, 256], dt.bfloat16, kind="ExternalInput").ap()
    out_ap = nc.dram_tensor("out", [b_sh], dt.float32, kind="ExternalOutput").ap()

    with tile.TileContext(nc) as tc:
        with (
            tc.tile_pool(name="singles", bufs=1) as singles,
            tc.tile_pool(name="xs", bufs=2) as xs_pool,
            tc.tile_pool(name="primt", bufs=2) as primt_pool,
            tc.tile_pool(name="batch", bufs=1) as bpool,
            tc.tile_pool(name="tmp", bufs=2) as tmp_pool,
            tc.tile_pool(name="sm", bufs=8) as sm_pool,
            tc.tile_pool(name="psum_p", bufs=2, space="PSUM") as psp_pool,
            tc.tile_pool(name="psum_u", bufs=3, space="PSUM") as psu_pool,
            tc.tile_pool(name="psum_t", bufs=2, space="PSUM") as pst_pool,
            tc.tile_pool(name="psum_w", bufs=1, space="PSUM") as psw_pool,
        ):
            w_sb = singles.tile([128, NH, 128], dt.float8e4)
            nc.sync.dma_start(out=w_sb[:], in_=w_ap[:, :, :])
            caps_sb = singles.tile([128, NUM_OBJ, 128], dt.bfloat16)
            nc.sync.dma_start(out=caps_sb[:], in_=caps_ap[:, :, :])
            capsum_sb = singles.tile([128, CAP_DIM], dt.bfloat16)
            nc.sync.dma_start(out=capsum_sb[:], in_=capsum_ap[:, :])
            bias_sb = singles.tile([1, 256], dt.bfloat16)
            nc.sync.dma_start(out=bias_sb[:], in_=bias_ap[:, :])
            magic_sb = singles.tile([128, 1], dt.uint32)
            nc.vector.memset(magic_sb[:], SQRT_MAGIC)
            out_sb = singles.tile([128, NCH], dt.float32)
            warm_sb = singles.tile([128, 2, SUP], dt.float8e4)
            nc.vector.memset(warm_sb.rearrange("p a b -> p (a b)"), 0)

            uh_all, t_all, b_all = {}, {}, {}
            for bi, chs in enumerate(batches):
                K = len(chs)
                uh_all[bi] = bpool.tile(
                    [128, K, N, D], dt.bfloat16, tag=f"uh{bi}", name=f"uh{bi}"
                )
                t_all[bi] = bpool.tile(
                    [128, K, D], dt.float32, tag=f"t{bi}", name=f"t{bi}"
                )
                b_all[bi] = bpool.tile(
                    [128, K, N], dt.float32, tag=f"b{bi}", name=f"b{bi}"
                )

            def smt(K, tag, dtype=dt.float32):
                return sm_pool.tile([128, K], dtype, tag=tag, name=tag)

            def sqrt_half(q, K):
                """bit-trick sqrt seed; error washes out through squash."""
                qu = q.bitcast(dt.uint32)
                s1 = smt(K, "sq1", dt.uint32)
                nc.vector.tensor_single_scalar(
                    s1[:], qu, 1, op=OP.logical_shift_right
                )
                s2 = smt(K, "sq2", dt.uint32)
                nc.vector.tensor_tensor(
                    s2[:],
                    s1[:],
                    _ap(magic_sb[:], [magic_sb[:].ap[0], [0, K]]),
                    op=OP.add,
                )
                return s2.bitcast(dt.float32)  # ~3.5% sqrt approx (validated)

            def tree_n(tag, src, K, dst):
                """[128,K,32,D] bf16 -> dst [128,K,D] fp32 via halving adds."""
                cur = src
                w = N
                with nc.allow_low_precision(reason="tree bf16 validated"):
                    while w > 2:
                        w //= 2
                        nxt = tmp_pool.tile(
                            [128, K, w, D], dt.bfloat16, tag=f"{tag}{w}",
                            name=f"{tag}{w}",
                        )
                        nc.vector.tensor_tensor(
                            nxt[:], cur[:, :, 0:w, :], cur[:, :, w : 2 * w, :],
                            op=OP.add,
                        )
                        cur = nxt
                nc.vector.tensor_tensor(
                    dst, cur[:, :, 0, :], cur[:, :, 1, :], op=OP.add
                )

            def tree_d(tag, src, K, dst, engine):
                """[128,K,N,16] bf16 -> dst [128,K,N] bf16 via halving adds."""
                cur = src
                w = D
                with nc.allow_low_precision(reason="tree bf16 validated"):
                    while w > 2:
                        w //= 2
                        nxt = tmp_pool.tile(
                            [128, K, N, w], dt.bfloat16, tag=f"{tag}{w}",
                            name=f"{tag}{w}",
                        )
                        engine.tensor_tensor(
                            nxt[:], cur[:, :, :, 0:w], cur[:, :, :, w : 2 * w],
                            op=OP.add,
                        )
                        cur = nxt
                    engine.tensor_tensor(
                        dst, cur[:, :, :, 0], cur[:, :, :, 1], op=OP.add
                    )

            def routing_batch(bi):
                chs = batches[bi]
                K = len(chs)
                uh = uh_all[bi]
                tt = t_all[bi]
                for r in range(3):
                    if r > 0:
                        if r == 2:
                            # r2 logits can reach ~56; subtract the max so
                            # se^2 stays in fp32 range. r1 logits are <~33
                            # (se^2 < 7e30), so r1 exps directly.
                            mx = smt(K, "mx")
                            nc.vector.tensor_reduce(
                                mx[:], b_all[bi][:], axis=AX.X, op=OP.max
                            )
                            bsub = sm_pool.tile(
                                [128, K, N], dt.float32, tag="bsub", name="bsub"
                            )
                            nc.vector.tensor_tensor(
                                bsub[:],
                                b_all[bi][:],
                                _ap(mx[:], [*mx[:].ap, [0, N]]),
                                op=OP.subtract,
                            )
                            esrc = bsub[:]
                        else:
                            esrc = b_all[bi][:]
                        e = sm_pool.tile(
                            [128, K, N], dt.bfloat16, tag="esm", name="esm"
                        )
                        nc.scalar.activation(e[:], esrc, AF.Exp)
                        se = smt(K, "se")
                        nc.vector.tensor_reduce(se[:], e[:], axis=AX.X, op=OP.add)
                        erep = tmp_pool.tile(
                            [128, K, N, D], dt.bfloat16, tag="erep", name="erep"
                        )
                        nc.scalar.copy(erep[:], _ap(e[:], [*e[:].ap, [0, D]]))
                        wmul = tmp_pool.tile(
                            [128, K, N, D], dt.bfloat16, tag="wmul", name="wmul"
                        )
                        nc.vector.tensor_tensor(
                            wmul.rearrange("p k n d -> p (k n d)"),
                            uh.rearrange("p k n d -> p (k n d)"),
                            erep.rearrange("p k n d -> p (k n d)"),
                            op=OP.mult,
                        )
                        tree_n("tn", wmul, K, tt[:])
                    # q = |t|^2, den = se^2 + q, rden = 1/den
                    sq = sm_pool.tile([128, K, D], dt.float32, tag="sqv", name="sqv")
                    nc.vector.tensor_tensor(sq[:], tt[:], tt[:], op=OP.mult)
                    q = smt(K, "q")
                    nc.vector.tensor_reduce(q[:], sq[:], axis=AX.X, op=OP.add)
                    den = smt(K, "den")
                    if r == 0:
                        nc.vector.tensor_single_scalar(
                            den[:], q[:], float(N * N), op=OP.add
                        )
                    else:
                        se2 = smt(K, "se2")
                        nc.vector.tensor_mul(se2[:], se[:], se[:])
                        nc.vector.tensor_add(den[:], q[:], se2[:])

                    rden = smt(K, "rden")
                    nc.vector.reciprocal(rden[:], den[:])
                    if r < 2:
                        sm = sqrt_half(q[:], K)
                        alpha2 = smt(K, "alpha2")
                        nc.vector.tensor_mul(alpha2[:], sm, rden[:])
                        amul = tmp_pool.tile(
                            [128, K, N, D], dt.bfloat16, tag="amul", name="amul"
                        )
                        if r == 0:
                            # GPSIMD multiply with broadcast t read (1x there
                            # anyway); frees DVE and skips materializing trep
                            tt3 = tt[:]
                            nc.gpsimd.tensor_tensor(
                                amul[:],
                                uh[:],
                                _ap(
                                    tt3,
                                    [tt3.ap[0], tt3.ap[1], [0, N], tt3.ap[2]],
                                ),
                                op=OP.mult,
                            )
                        else:
                            trep = tmp_pool.tile(
                                [128, K, N, D], dt.bfloat16, tag="trep",
                                name="trep",
                            )
                            tt3 = tt[:]
                            nc.scalar.copy(
                                trep[:],
                                _ap(
                                    tt3,
                                    [tt3.ap[0], tt3.ap[1], [0, N], tt3.ap[2]],
                                ),
                            )
                            nc.vector.tensor_tensor(
                                amul.rearrange("p k n d -> p (k n d)"),
                                uh.rearrange("p k n d -> p (k n d)"),
                                trep.rearrange("p k n d -> p (k n d)"),
                                op=OP.mult,
                            )
                        dta = sm_pool.tile(
                            [128, K, N], dt.bfloat16, tag="dta", name="dta"
                        )
                        tree_d("td", amul, K, dta[:], nc.vector)
                        if r == 0:
                            nc.vector.tensor_tensor(
                                b_all[bi][:],
                                dta[:],
                                _ap(alpha2[:], [*alpha2[:].ap, [0, N]]),
                                op=OP.mult,
                            )
                        else:
                            badd = sm_pool.tile(
                                [128, K, N], dt.float32, tag="badd", name="badd"
                            )
                            nc.vector.tensor_tensor(
                                badd[:],
                                dta[:],
                                _ap(alpha2[:], [*alpha2[:].ap, [0, N]]),
                                op=OP.mult,
                            )
                            nc.vector.tensor_tensor(
                                b_all[bi][:], b_all[bi][:], badd[:], op=OP.add
                            )
                    else:
                        nc.vector.tensor_mul(
                            out_sb[:, chs[0] : chs[0] + K], q[:], rden[:]
                        )
                        nc.sync.dma_start(
                            out=out_ap.rearrange("(c p) -> p c", p=128)[
                                :, chs[0] : chs[0] + K
                            ],
                            in_=out_sb[:, chs[0] : chs[0] + K],
                        )

            # PE p-state warmup while super 0 streams in
            psw = psw_pool.tile([128, SUP], dt.float32)
            for wi in range(24):
                nc.tensor.matmul(
                    psw[:],
                    warm_sb[:, 0, 0:128],
                    warm_sb[:, 1, :],
                    start=(wi == 0),
                    stop=(wi == 23),
                )

            for sp in range(NSUP):
                xs = xs_pool.tile([128, NH, SUP], dt.float8e4)
                for qd in range(NQ):
                    nc.gpsimd.dma_start(
                        out=xs[:, qd * HQ : (qd + 1) * HQ, :],
                        in_=xt_ap[sp, qd],
                    )
                psp = psp_pool.tile([128, SUP], dt.float32)
                # Linear bias rides as a K=1 bf16 matmul against ones
                ones_bc = _ap(
                    bias_sb[:, 128:256],
                    [bias_sb[:, 128:256].ap[0], [0, CPS], [1, 128]],
                )
                nc.tensor.matmul(
                    psp[:], bias_sb[:, 0:128], ones_bc, start=True, stop=False
                )
                for hp in range(NH // 2):
                    nc.tensor.matmul(
                        psp[:],
                        w_sb[:, 2 * hp : 2 * hp + 2, :],
                        xs[:, 2 * hp : 2 * hp + 2, :],
                        start=False,
                        stop=(hp == NH // 2 - 1),
                        perf_mode=PM.DoubleRow,
                    )
                primt = primt_pool.tile([128, SUP], dt.bfloat16)
                nc.scalar.copy(primt[:], psp[:])

                for c in range(CPS):
                    s = sp * CPS + c
                    bi = chunk_to_batch[s]
                    k = s - batches[bi][0]
                    lhsT = primt[:, c * 128 : (c + 1) * 128]
                    psu = psu_pool.tile([128, NUM_OBJ * 128], dt.float32)
                    nc.tensor.matmul(
                        psu[:],
                        lhsT,
                        caps_sb.rearrange("p o f -> p (o f)"),
                        start=True,
                        stop=True,
                    )
                    pst = pst_pool.tile([128, CAP_DIM], dt.float32)
                    nc.tensor.matmul(
                        pst[:], lhsT, capsum_sb[:], start=True, stop=True
                    )
                    nc.scalar.copy(
                        uh_all[bi][:, k, :, :].rearrange("p n d -> p (n d)"), psu[:]
                    )
                    nc.scalar.copy(t_all[bi][:, k, :], pst[:])

                    if s in last_chunk_to_batch:
                        routing_batch(last_chunk_to_batch[s])

    nc.compile()
    return nc


def _prep_params(W, b_lin, out_caps, hidden=HIDDEN):
    NH = hidden // 128
    w_f = np.ascontiguousarray(
        (W.astype(np.float32) * W_SCALE)
        .reshape(NH, 128, NUM_CAPS * CAP_DIM)
        .transpose(1, 0, 2)
    ).astype(FP8)
    caps_bd = np.zeros((NUM_OBJ, 128, 128), np.float32)
    for o in range(NUM_OBJ):
        for i in range(NUM_CAPS):
            caps_bd[
                o, i * CAP_DIM : (i + 1) * CAP_DIM, i * CAP_DIM : (i + 1) * CAP_DIM
            ] = out_caps[o, i]
    caps_bd /= W_SCALE
    capsum = caps_bd.sum(0)
    caps_bd = np.ascontiguousarray(caps_bd.transpose(1, 0, 2)).astype(BF16)
    capsum_t0 = np.zeros((128, CAP_DIM), np.float32)
    for i in range(NUM_CAPS):
        capsum_t0[i * CAP_DIM : (i + 1) * CAP_DIM, :] = capsum[
            i * CAP_DIM : (i + 1) * CAP_DIM, i * CAP_DIM : (i + 1) * CAP_DIM
        ]
    bias_row = np.concatenate(
        [
            b_lin.astype(np.float32).reshape(1, 128) * W_SCALE,
            np.ones((1, 128), np.float32),
        ],
        axis=1,
    ).astype(BF16)
    return w_f, caps_bd, np.ascontiguousarray(capsum_t0).astype(BF16), bias_row


_NC_CACHE = {}


def kernel(x, W, b_lin, out_caps):
    global LAST_EXEC_TIME_NS
    from concourse.bass_utils import run_bass_kernel_spmd

    x = np.asarray(x)
    W = np.asarray(W)
    b_lin = np.asarray(b_lin)
    out_caps = np.asarray(out_caps)
    bsz, hidden = x.shape
    b_sh = bsz // N_CORES
    NH = hidden // 128
    SUP = 512
    NSUP = b_sh // SUP
    HQ = NH // NQ

    key = (hidden, b_sh)
    if key not in _NC_CACHE:
        _NC_CACHE[key] = build_bass(hidden=hidden, b_sh=b_sh)
    nc = _NC_CACHE[key]

    w_f, caps_bd, capsum_t0, bias_row = _prep_params(W, b_lin, out_caps, hidden)

    in_maps = []
    for i in range(N_CORES):
        shard = x[i * b_sh : (i + 1) * b_sh]
        # [sp, qd, p, hcq, b]: every DMA issue reads contiguous DRAM
        xt = np.ascontiguousarray(
            shard.reshape(NSUP, SUP, NQ, HQ, 128).transpose(0, 2, 4, 3, 1)
        ).astype(FP8)
        in_maps.append(
            {
                "xt": xt,
                "w": w_f,
                "caps": caps_bd,
                "capsum": capsum_t0,
                "bias": bias_row,
            }
        )

    res = run_bass_kernel_spmd(
        nc,
        in_maps,
        core_ids=list(range(N_CORES)),
        trace=bool(int(os.environ.get("BASS_TRACE", "0") or "0")),
    )
    LAST_EXEC_TIME_NS = res.exec_time_ns
    return np.concatenate([res.results[i]["out"] for i in range(N_CORES)])


# revision 17
# speedup vs baseline: 1.3831x; 1.0136x over previous
"""CapsuleRewardHead Trainium2 kernel (8-core data parallel).

Math (per batch row b):
    primary = x @ W + b_lin                    [B, 128]  (128 = 8 caps x 16 dim)
    u_hat[b,o,i,j] = sum_c primary[b,i,c] * out_caps[o,i,c,j]
    3 rounds of dynamic routing over N=32 capsule pairs (o,i), D=16
    out[b] = |squash(s_final)|

Device strategy per core (2048 batch rows):
  - host: quantize x shard to fp8 e4m3 and pre-tile to [sp][queue][128 part]
    [hc][b] so every DMA issue reads one fully contiguous DRAM block; W to
    fp8 scaled by 1024 (descaled via caps/capsum); W/caps pre-transposed so
    their loads are contiguous too. Linear bias rides as a K=1 bf16 matmul.
  - MM1 (PE): DoubleRow fp8 matmuls contract h-chunk PAIRS (256 rows/pass)
    into PSUM: primaryT[ic, b] per 512-col super.
  - MM2 (PE): u_hat[b, (o,i,j)] via block-diagonal capsule matrices straight
    into routing layout [128b, 512]; an extra N=16 matmul against
    sum_o(caps) yields round-0's uniform-coefficient sum t0 for free.
  - routing engine split: softmax exp emits compact [K,N] on ACT; ACT also
    materializes the e/t broadcasts so the big multiplies run in DVE 2x
    bf16 mode; round-0's agreement multiply goes to GPSIMD with a
    broadcast t read (GPSIMD is 1x anyway). The n/d reductions run as
    pairwise halving trees of 2x-mode tensor_tensor adds (tensor_reduce
    has no fast mode; trees are ~2x faster). sqrt via bit-trick seed;
    unnormalized accumulators (q = |t|^2, se = sum e) keep the per-round
    scalar chain short.
"""

import os

import numpy as np
import ml_dtypes

B = 16384
HIDDEN = 4096
NUM_OBJ = 4
NUM_CAPS = 8
CAP_DIM = 16
N_ROUTE = 32  # NUM_OBJ * NUM_CAPS
N_CORES = 8

LAST_EXEC_TIME_NS = None  # set after each run when BASS_TRACE=1

BF16 = ml_dtypes.bfloat16
FP8 = ml_dtypes.float8_e4m3
W_SCALE = 1024.0
SQRT_MAGIC = 0x1FBD1DF5
NQ = 4  # x sub-DMA issues per super (4KB descriptors)


def _ap(ap, dims):
    import concourse.bass as bass

    return bass.AP(tensor=ap.tensor, offset=ap.offset, ap=dims)


def build_bass(hidden=HIDDEN, b_sh=B // N_CORES, batch_plan=(6, 4, 3, 2, 1)):
    import concourse.tile as tile
    from concourse import bacc, mybir

    NH = hidden // 128
    NCH = b_sh // 128  # 128-row chunks
    SUP = 512
    NSUP = b_sh // SUP
    CPS = SUP // 128
    HQ = NH // NQ
    assert sum(batch_plan) == NCH
    N, D = N_ROUTE, CAP_DIM
    dt = mybir.dt
    AX = mybir.AxisListType
    OP = mybir.AluOpType
    AF = mybir.ActivationFunctionType
    PM = mybir.MatmulPerfMode

    batches = []
    pos = 0
    for k in batch_plan:
        batches.append(list(range(pos, pos + k)))
        pos += k
    last_chunk_to_batch = {b[-1]: bi for bi, b in enumerate(batches)}
    chunk_to_batch = {}
    for bi, chs in enumerate(batches):
        for ch in chs:
            chunk_to_batch[ch] = bi

    nc = bacc.Bacc("TRN2", target_bir_lowering=False, debug=False, num_devices=N_CORES)

    xt_ap = nc.dram_tensor(
        "xt", [NSUP, NQ, 128, HQ, SUP], dt.float8e4, kind="ExternalInput"
    ).ap()
    w_ap = nc.dram_tensor("w", [128, NH, 128], dt.float8e4, kind="ExternalInput").ap()
    caps_ap = nc.dram_tensor(
        "caps", [128, NUM_OBJ, 128], dt.bfloat16, kind="ExternalInput"
    ).ap()
    capsum_ap = nc.dram_tensor(
        "capsum", [128, CAP_DIM], dt.bfloat16, kind="ExternalInput"
    ).ap()
    bias_ap = nc.dram_tensor("bias", [1, 256], dt.bfloat16, kind="ExternalInput").ap()
    out_ap = nc.dram_tensor("out", [b_sh], dt.float32, kind="ExternalOutput").ap()

    with tile.TileContext(nc) as tc:
        with (
            tc.tile_pool(name="singles", bufs=1) as singles,
            tc.tile_pool(name="xs", bufs=2) as xs_pool,
            tc.tile_pool(name="primt", bufs=2) as primt_pool,
            tc.tile_pool(name="batch", bufs=1) as bpool,
            tc.tile_pool(name="tmp", bufs=2) as tmp_pool,
            tc.tile_pool(name="sm", bufs=8) as sm_pool,
            tc.tile_pool(name="psum_p", bufs=2, space="PSUM") as psp_pool,
            tc.tile_pool(name="psum_u", bufs=3, space="PSUM") as psu_pool,
            tc.tile_pool(name="psum_t", bufs=2, space="PSUM") as pst_pool,
            tc.tile_pool(name="psum_w", bufs=1, space="PSUM") as psw_pool,
        ):
            w_sb = singles.tile([128, NH, 128], dt.float8e4)
            nc.sync.dma_start(out=w_sb[:], in_=w_ap[:, :, :])
            caps_sb = singles.tile([128, NUM_OBJ, 128], dt.bfloat16)
            nc.sync.dma_start(out=caps_sb[:], in_=caps_ap[:, :, :])
            capsum_sb = singles.tile([128, CAP_DIM], dt.bfloat16)
            nc.sync.dma_start(out=capsum_sb[:], in_=capsum_ap[:, :])
            bias_sb = singles.tile([1, 256], dt.bfloat16)
            nc.sync.dma_start(out=bias_sb[:], in_=bias_ap[:, :])
            magic_sb = singles.tile([128, 1], dt.uint32)
            nc.vector.memset(magic_sb[:], SQRT_MAGIC)
            out_sb = singles.tile([128, NCH], dt.float32)
            warm_sb = singles.tile([128, 2, SUP], dt.float8e4)
            nc.vector.memset(warm_sb.rearrange("p a b -> p (a b)"), 0)

            uh_all, t_all, b_all = {}, {}, {}
            for bi, chs in enumerate(batches):
                K = len(chs)
                # [N, K, D]: n outermost so n-halving tree operands are
                # fully contiguous 1D (hardware 2x mode needs that)
                uh_all[bi] = bpool.tile(
                    [128, N, K, D], dt.bfloat16, tag=f"uh{bi}", name=f"uh{bi}"
                )
                t_all[bi] = bpool.tile(
                    [128, K, D], dt.float32, tag=f"t{bi}", name=f"t{bi}"
                )
                b_all[bi] = bpool.tile(
                    [128, K, N], dt.float32, tag=f"b{bi}", name=f"b{bi}"
                )

            def smt(K, tag, dtype=dt.float32):
                return sm_pool.tile([128, K], dtype, tag=tag, name=tag)

            def sqrt_half(q, K):
                """bit-trick sqrt seed; error washes out through squash."""
                qu = q.bitcast(dt.uint32)
                s1 = smt(K, "sq1", dt.uint32)
                nc.vector.tensor_single_scalar(
                    s1[:], qu, 1, op=OP.logical_shift_right
                )
                s2 = smt(K, "sq2", dt.uint32)
                nc.vector.tensor_tensor(
                    s2[:],
                    s1[:],
                    _ap(magic_sb[:], [magic_sb[:].ap[0], [0, K]]),
                    op=OP.add,
                )
                return s2.bitcast(dt.float32)  # ~3.5% sqrt approx (validated)

            def tree_n(tag, src, K, dst):
                """[128,N,K,D] bf16 -> dst [128,K,D] fp32 via halving adds.

                Halves along outermost n are contiguous 1D blocks -> 2x mode.
                """
                cur = src
                w = N
                with nc.allow_low_precision(reason="tree bf16 validated"):
                    while w > 2:
                        w //= 2
                        nxt = tmp_pool.tile(
                            [128, w, K, D], dt.bfloat16, tag=f"{tag}{w}",
                            name=f"{tag}{w}",
                        )
                        nc.vector.tensor_tensor(
                            nxt[:], cur[:, 0:w, :, :], cur[:, w : 2 * w, :, :],
                            op=OP.add,
                        )
                        cur = nxt
                nc.vector.tensor_tensor(
                    dst, cur[:, 0, :, :], cur[:, 1, :, :], op=OP.add
                )

            def routing_batch(bi):
                chs = batches[bi]
                K = len(chs)
                uh = uh_all[bi]
                tt = t_all[bi]
                for r in range(3):
                    if r > 0:
                        if r == 2:
                            # r2 logits can reach ~56; subtract the max so
                            # se^2 stays in fp32 range. r1 logits are <~33
                            # (se^2 < 7e30), so r1 exps directly.
                            mx = smt(K, "mx")
                            nc.vector.tensor_reduce(
                                mx[:], b_all[bi][:], axis=AX.X, op=OP.max
                            )
                            bsub = sm_pool.tile(
                                [128, K, N], dt.float32, tag="bsub", name="bsub"
                            )
                            nc.vector.tensor_tensor(
                                bsub[:],
                                b_all[bi][:],
                                _ap(mx[:], [*mx[:].ap, [0, N]]),
                                op=OP.subtract,
                            )
                            esrc = bsub[:]
                        else:
                            esrc = b_all[bi][:]
                        e = sm_pool.tile(
                            [128, K, N], dt.bfloat16, tag="esm", name="esm"
                        )
                        nc.scalar.activation(e[:], esrc, AF.Exp)
                        se = smt(K, "se")
                        nc.vector.tensor_reduce(se[:], e[:], axis=AX.X, op=OP.add)
                        erep = tmp_pool.tile(
                            [128, N, K, D], dt.bfloat16, tag="erep", name="erep"
                        )
                        e3 = e[:]
                        nc.scalar.copy(
                            erep[:], _ap(e3, [e3.ap[0], [1, N], [N, K], [0, D]])
                        )
                        wmul = tmp_pool.tile(
                            [128, N, K, D], dt.bfloat16, tag="wmul", name="wmul"
                        )
                        nc.vector.tensor_tensor(
                            wmul.rearrange("p n k d -> p (n k d)"),
                            uh.rearrange("p n k d -> p (n k d)"),
                            erep.rearrange("p n k d -> p (n k d)"),
                            op=OP.mult,
                        )
                        tree_n("tn", wmul, K, tt[:])
                    # q = |t|^2, den = se^2 + q, rden = 1/den
                    sq = sm_pool.tile([128, K, D], dt.float32, tag="sqv", name="sqv")
                    nc.vector.tensor_tensor(sq[:], tt[:], tt[:], op=OP.mult)
                    q = smt(K, "q")
                    nc.vector.tensor_reduce(q[:], sq[:], axis=AX.X, op=OP.add)
                    den = smt(K, "den")
                    if r == 0:
                        nc.vector.tensor_single_scalar(
                            den[:], q[:], float(N * N), op=OP.add
                        )
                    else:
                        se2 = smt(K, "se2")
                        nc.vector.tensor_mul(se2[:], se[:], se[:])
                        nc.vector.tensor_add(den[:], q[:], se2[:])

                    rden = smt(K, "rden")
                    nc.vector.reciprocal(rden[:], den[:])
                    if r < 2:
                        sm = sqrt_half(q[:], K)
                        alpha2 = smt(K, "alpha2")
                        nc.vector.tensor_mul(alpha2[:], sm, rden[:])
                        amul = tmp_pool.tile(
                            [128, N, K, D], dt.bfloat16, tag="amul", name="amul"
                        )
                        tt3 = tt[:]
                        t_bc = _ap(tt3, [tt3.ap[0], [0, N], [D, K], [1, D]])
                        if r == 0:
                            # GPSIMD multiply with broadcast t read (1x there
                            # anyway); frees DVE and skips materializing trep
                            nc.gpsimd.tensor_tensor(
                                amul[:], uh[:], t_bc, op=OP.mult
                            )
                        else:
                            trep = tmp_pool.tile(
                                [128, N, K, D], dt.bfloat16, tag="trep",
                                name="trep",
                            )
                            nc.scalar.copy(trep[:], t_bc)
                            nc.vector.tensor_tensor(
                                amul.rearrange("p n k d -> p (n k d)"),
                                uh.rearrange("p n k d -> p (n k d)"),
                                trep.rearrange("p n k d -> p (n k d)"),
                                op=OP.mult,
                            )
                        dta = sm_pool.tile(
                            [128, N, K], dt.bfloat16, tag="dta", name="dta"
                        )
                        with nc.allow_low_precision(reason="dta bf16 validated"):
                            nc.vector.tensor_reduce(
                                dta[:], amul[:], axis=AX.X, op=OP.add
                            )
                        # dta is [n, k]; read it back transposed for the
                        # [k, n]-ordered logit update
                        d3 = dta[:]
                        dta_kn = _ap(d3, [d3.ap[0], [1, K], [K, N]])
                        if r == 0:
                            nc.vector.tensor_tensor(
                                b_all[bi][:],
                                dta_kn,
                                _ap(alpha2[:], [*alpha2[:].ap, [0, N]]),
                                op=OP.mult,
                            )
                        else:
                            badd = sm_pool.tile(
                                [128, K, N], dt.float32, tag="badd", name="badd"
                            )
                            nc.vector.tensor_tensor(
                                badd[:],
                                dta_kn,
                                _ap(alpha2[:], [*alpha2[:].ap, [0, N]]),
                                op=OP.mult,
                            )
                            nc.vector.tensor_tensor(
                                b_all[bi][:], b_all[bi][:], badd[:], op=OP.add
                            )
                    else:
                        nc.vector.tensor_mul(
                            out_sb[:, chs[0] : chs[0] + K], q[:], rden[:]
                        )
                        nc.sync.dma_start(
                            out=out_ap.rearrange("(c p) -> p c", p=128)[
                                :, chs[0] : chs[0] + K
                            ],
                            in_=out_sb[:, chs[0] : chs[0] + K],
                        )

            # PE p-state warmup while super 0 streams in
            psw = psw_pool.tile([128, SUP], dt.float32)
            for wi in range(24):
                nc.tensor.matmul(
                    psw[:],
                    warm_sb[:, 0, 0:128],
                    warm_sb[:, 1, :],
                    start=(wi == 0),
                    stop=(wi == 23),
                )

            for sp in range(NSUP):
                xs = xs_pool.tile([128, NH, SUP], dt.float8e4)
                for qd in range(NQ):
                    nc.sync.dma_start(
                        out=xs[:, qd * HQ : (qd + 1) * HQ, :],
                        in_=xt_ap[sp, qd],
                    )
                psp = psp_pool.tile([128, SUP], dt.float32)
                # Linear bias rides as a K=1 bf16 matmul against ones
                ones_bc = _ap(
                    bias_sb[:, 128:256],
                    [bias_sb[:, 128:256].ap[0], [0, CPS], [1, 128]],
                )
                nc.tensor.matmul(
                    psp[:], bias_sb[:, 0:128], ones_bc, start=True, stop=False
                )
                for hp in range(NH // 2):
                    nc.tensor.matmul(
                        psp[:],
                        w_sb[:, 2 * hp : 2 * hp + 2, :],
                        xs[:, 2 * hp : 2 * hp + 2, :],
                        start=False,
                        stop=(hp == NH // 2 - 1),
                        perf_mode=PM.DoubleRow,
                    )
                primt = primt_pool.tile([128, SUP], dt.bfloat16)
                nc.scalar.copy(primt[:], psp[:])

                for c in range(CPS):
                    s = sp * CPS + c
                    bi = chunk_to_batch[s]
                    k = s - batches[bi][0]
                    lhsT = primt[:, c * 128 : (c + 1) * 128]
                    psu = psu_pool.tile([128, NUM_OBJ * 128], dt.float32)
                    nc.tensor.matmul(
                        psu[:],
                        lhsT,
                        caps_sb.rearrange("p o f -> p (o f)"),
                        start=True,
                        stop=True,
                    )
                    pst = pst_pool.tile([128, CAP_DIM], dt.float32)
                    nc.tensor.matmul(
                        pst[:], lhsT, capsum_sb[:], start=True, stop=True
                    )
                    nc.scalar.copy(
                        uh_all[bi][:, :, k, :],
                        psu.rearrange("p (n d) -> p n d", n=N),
                    )
                    nc.scalar.copy(t_all[bi][:, k, :], pst[:])

                    if s in last_chunk_to_batch:
                        routing_batch(last_chunk_to_batch[s])

    nc.compile()
    return nc


def _prep_params(W, b_lin, out_caps, hidden=HIDDEN):
    NH = hidden // 128
    w_f = np.ascontiguousarray(
        (W.astype(np.float32) * W_SCALE)
        .reshape(NH, 128, NUM_CAPS * CAP_DIM)
        .transpose(1, 0, 2)
    ).astype(FP8)
    caps_bd = np.zeros((NUM_OBJ, 128, 128), np.float32)
    for o in range(NUM_OBJ):
        for i in range(NUM_CAPS):
            caps_bd[
                o, i * CAP_DIM : (i + 1) * CAP_DIM, i * CAP_DIM : (i + 1) * CAP_DIM
            ] = out_caps[o, i]
    caps_bd /= W_SCALE
    capsum = caps_bd.sum(0)
    caps_bd = np.ascontiguousarray(caps_bd.transpose(1, 0, 2)).astype(BF16)
    capsum_t0 = np.zeros((128, CAP_DIM), np.float32)
    for i in range(NUM_CAPS):
        capsum_t0[i * CAP_DIM : (i + 1) * CAP_DIM, :] = capsum[
            i * CAP_DIM : (i + 1) * CAP_DIM, i * CAP_DIM : (i + 1) * CAP_DIM
        ]
    bias_row = np.concatenate(
        [
            b_lin.astype(np.float32).reshape(1, 128) * W_SCALE,
            np.ones((1, 128), np.float32),
        ],
        axis=1,
    ).astype(BF16)
    return w_f, caps_bd, np.ascontiguousarray(capsum_t0).astype(BF16), bias_row


_NC_CACHE = {}


def kernel(x, W, b_lin, out_caps):
    global LAST_EXEC_TIME_NS
    from concourse.bass_utils import run_bass_kernel_spmd

    x = np.asarray(x)
    W = np.asarray(W)
    b_lin = np.asarray(b_lin)
    out_caps = np.asarray(out_caps)
    bsz, hidden = x.shape
    b_sh = bsz // N_CORES
    NH = hidden // 128
    SUP = 512
    NSUP = b_sh // SUP
    HQ = NH // NQ

    key = (hidden, b_sh)
    if key not in _NC_CACHE:
        _NC_CACHE[key] = build_bass(hidden=hidden, b_sh=b_sh)
    nc = _NC_CACHE[key]

    w_f, caps_bd, capsum_t0, bias_row = _prep_params(W, b_lin, out_caps, hidden)

    in_maps = []
    for i in range(N_CORES):
        shard = x[i * b_sh : (i + 1) * b_sh]
        # [sp, qd, p, hcq, b]: every DMA issue reads contiguous DRAM
        xt = np.ascontiguousarray(
            shard.reshape(NSUP, SUP, NQ, HQ, 128).transpose(0, 2, 4, 3, 1)
        ).astype(FP8)
        in_maps.append(
            {
                "xt": xt,
                "w": w_f,
                "caps": caps_bd,
                "capsum": capsum_t0,
                "bias": bias_row,
            }
        )

    res = run_bass_kernel_spmd(
        nc,
        in_maps,
        core_ids=list(range(N_CORES)),
        trace=bool(int(os.environ.get("BASS_TRACE", "0") or "0")),
    )
    LAST_EXEC_TIME_NS = res.exec_time_ns
    return np.concatenate([res.results[i]["out"] for i in range(N_CORES)])
